# revision 3
# baseline (speedup 1.0000x reference)
"""CapsuleRewardHead Trainium2 kernel (8-core data parallel).

Math (per batch row b):
    primary = x @ W + b_lin                    [B, 128]  (128 = 8 caps x 16 dim)
    u_hat[b,o,i,j] = sum_c primary[b,i,c] * out_caps[o,i,c,j]
    3 rounds of dynamic routing over N=32 capsule pairs (o,i), D=16
    out[b] = |squash(s_final)|

Device strategy per core (2048 batch rows):
  - host pre-converts the x shard to bf16 in a super-contiguous layout
    (one 8KB DRAM line per (super, h-slice, partition)), halving HBM
    traffic vs fp32 and keeping DMA descriptor-lean. 16 h-sliced loads
    are issued from the Sync engine so the stream is never queued behind
    data-dependent work.
  - MM1 (PE): primaryT[ic, b] += W.T @ xT over 32 h-chunks into PSUM;
    the Linear bias rides as a K=1 matmul against ones. Zero-weight
    filler matmuls pad PE idle gaps so the clock stays at full p-state
    (a cold/idle PE runs 512-col matmuls at ~427ns instead of ~216ns).
  - MM2 (PE): three matmuls per 128-row chunk produce u_hat in BOTH
    (n,d) and (d,n) layouts plus the round-0 uniform sum t0 (via
    column-permuted / summed copies of the block-diag capsule matrix),
    so every routing reduce is a unit-stride innermost-axis DVE reduce.
  - routing: e = exp(b) is stored compact [p,k,n] (bf16) and broadcast
    into the weighted-sum multiply via a 0-stride middle axis; t is
    kept bf16 so the agreement multiply's broadcast operand preserves
    DVE 2x mode. Unnormalized accumulators (q = |t|^2, se = sum e):
    alpha = sqrt(q)/(se^2+q), out = q/(se^2+q); sqrt via bit-trick.
    Engine split: DVE = big mults+reduces, ACT = exp + uh/t0/primt
    drains, GpSimd = uhT drains + logit updates + output DMA.
"""

import os

import numpy as np
import ml_dtypes

B = 16384
HIDDEN = 4096
NUM_OBJ = 4
NUM_CAPS = 8
CAP_DIM = 16
N_ROUTE = 32  # NUM_OBJ * NUM_CAPS
N_CORES = 8

LAST_EXEC_TIME_NS = None  # set after each run when BASS_TRACE=1

BF16 = ml_dtypes.bfloat16
SQRT_MAGIC = 0x1FBD1DF5


def _ap(ap, dims):
    import concourse.bass as bass

    return bass.AP(tensor=ap.tensor, offset=ap.offset, ap=dims)


def build_bass(
    hidden=HIDDEN,
    b_sh=B // N_CORES,
    batch_plan=(4, 4, 4, 2, 1, 1),
    n_warm=16,
    n_fill=3,
):
    import concourse.tile as tile
    from concourse import bacc, mybir

    NH = hidden // 128  # 32 h-chunks
    SUP = 512
    NSP = b_sh // SUP  # supers
    CPS = SUP // 128  # chunks per super
    NCH = b_sh // 128  # 16 chunks
    NQ = 4  # h-slices per super
    HQ = NH // NQ
    N, D = N_ROUTE, CAP_DIM
    dt = mybir.dt
    AX = mybir.AxisListType
    OP = mybir.AluOpType
    AF = mybir.ActivationFunctionType
    assert sum(batch_plan) == NCH

    batches = []
    pos = 0
    for k in batch_plan:
        batches.append(list(range(pos, pos + k)))
        pos += k
    last_chunk_to_batch = {b[-1]: bi for bi, b in enumerate(batches)}
    chunk_to_batch = {}
    for bi, chs in enumerate(batches):
        for ch in chs:
            chunk_to_batch[ch] = bi

    nc = bacc.Bacc("TRN2", target_bir_lowering=False, debug=False, num_devices=N_CORES)

    xt_ap = nc.dram_tensor(
        "xt", [NSP, NQ, 128, HQ * SUP], dt.bfloat16, kind="ExternalInput"
    ).ap()
    w_ap = nc.dram_tensor("w", [128, NH * 128], dt.bfloat16, kind="ExternalInput").ap()
    capsn_ap = nc.dram_tensor(
        "capsn", [128, NUM_OBJ * 128], dt.bfloat16, kind="ExternalInput"
    ).ap()
    capst_ap = nc.dram_tensor(
        "capst", [128, D * N], dt.bfloat16, kind="ExternalInput"
    ).ap()
    capsum_ap = nc.dram_tensor(
        "capsum", [128, D], dt.bfloat16, kind="ExternalInput"
    ).ap()
    bias_ap = nc.dram_tensor("bias", [1, 256], dt.bfloat16, kind="ExternalInput").ap()
    out_ap = nc.dram_tensor("out", [128, NCH], dt.float32, kind="ExternalOutput").ap()

    with tile.TileContext(nc) as tc:
        with (
            tc.tile_pool(name="singles", bufs=1) as singles,
            tc.tile_pool(name="xs", bufs=3) as xs_pool,
            tc.tile_pool(name="primt", bufs=2) as primt_pool,
            tc.tile_pool(name="batch", bufs=1) as bpool,
            tc.tile_pool(name="tmp", bufs=2) as tmp_pool,
            tc.tile_pool(name="sm", bufs=8) as sm_pool,
            tc.tile_pool(name="psum_p", bufs=2, space="PSUM") as psp_pool,
            tc.tile_pool(name="psum_a", bufs=2, space="PSUM") as psa_pool,
            tc.tile_pool(name="psum_b", bufs=2, space="PSUM") as psb_pool,
            tc.tile_pool(name="psum_c", bufs=2, space="PSUM") as psc_pool,
        ):
            w_sb = singles.tile([128, NH, 128], dt.bfloat16)
            nc.sync.dma_start(out=w_sb.rearrange("p h f -> p (h f)"), in_=w_ap[:, :])
            capsn_sb = singles.tile([128, NUM_OBJ * 128], dt.bfloat16)
            nc.sync.dma_start(out=capsn_sb[:], in_=capsn_ap[:, :])
            capst_sb = singles.tile([128, D * N], dt.bfloat16)
            nc.sync.dma_start(out=capst_sb[:], in_=capst_ap[:, :])
            capsum_sb = singles.tile([128, D], dt.bfloat16)
            nc.sync.dma_start(out=capsum_sb[:], in_=capsum_ap[:, :])
            bias_sb = singles.tile([1, 256], dt.bfloat16)
            nc.sync.dma_start(out=bias_sb[:], in_=bias_ap[:, :])
            zeros_sb = singles.tile([128, 128], dt.bfloat16)
            nc.vector.memset(zeros_sb[:], 0)
            magic_sb = singles.tile([128, 1], dt.uint32)
            nc.vector.memset(magic_sb[:], SQRT_MAGIC)
            out_sb = singles.tile([128, NCH], dt.float32)

            w4 = w_sb.rearrange("p h f -> p (h f)")[:, 0:512]

            uh_all, uht_all, t_all, b_all = {}, {}, {}, {}
            for bi, chs in enumerate(batches):
                K = len(chs)
                uh_all[bi] = bpool.tile(
                    [128, K, N, D], dt.bfloat16, tag=f"uh{bi}", name=f"uh{bi}"
                )
                uht_all[bi] = bpool.tile(
                    [128, K, D, N], dt.bfloat16, tag=f"uht{bi}", name=f"uht{bi}"
                )
                t_all[bi] = bpool.tile(
                    [128, K, D], dt.bfloat16, tag=f"t{bi}", name=f"t{bi}"
                )
                b_all[bi] = bpool.tile(
                    [128, K, N], dt.float32, tag=f"b{bi}", name=f"b{bi}"
                )

            def smt(K, tag, dtype=dt.float32):
                return sm_pool.tile([128, K], dtype, tag=tag, name=tag)

            def sqrt_half(q, K):
                """bit-trick sqrt seed; error washes out through squash."""
                qu = q.bitcast(dt.uint32)
                s1 = smt(K, "sq1", dt.uint32)
                nc.vector.tensor_single_scalar(
                    s1[:], qu, 1, op=OP.logical_shift_right
                )
                s2 = smt(K, "sq2", dt.uint32)
                nc.vector.tensor_tensor(
                    s2[:],
                    s1[:],
                    _ap(magic_sb[:], [magic_sb[:].ap[0], [0, K]]),
                    op=OP.add,
                )
                return s2.bitcast(dt.float32)  # ~3.5% sqrt approx (validated)

            def routing_batch(bi):
                chs = batches[bi]
                K = len(chs)
                uh = uh_all[bi]
                uht = uht_all[bi]
                tt = t_all[bi]
                bb = b_all[bi]
                se = None
                for r in range(3):
                    if r > 0:
                        if r == 2:
                            # r2 logits can reach ~56; subtract the max so
                            # se^2 stays in fp32 range. r1 logits are <~33
                            # (se^2 < 7e30), so r1 exps directly.
                            mx = smt(K, "mx")
                            nc.vector.tensor_reduce(
                                mx[:], bb[:], axis=AX.X, op=OP.max
                            )
                            bsub = sm_pool.tile(
                                [128, K, N], dt.float32, tag="bsub", name="bsub"
                            )
                            nc.gpsimd.tensor_tensor(
                                bsub[:],
                                bb[:],
                                _ap(mx[:], [*mx[:].ap, [0, N]]),
                                op=OP.subtract,
                            )
                            esrc = bsub[:]
                        else:
                            esrc = bb[:]
                        ee = sm_pool.tile(
                            [128, K, N], dt.bfloat16, tag="ee", name="ee"
                        )
                        nc.scalar.activation(ee[:], esrc, AF.Exp)
                        se = smt(K, "se")
                        nc.vector.tensor_reduce(se[:], ee[:], axis=AX.X, op=OP.add)
                        wm = tmp_pool.tile(
                            [128, K, D, N], dt.bfloat16, tag="wm", name="wm"
                        )
                        eb = _ap(
                            ee[:], [ee[:].ap[0], ee[:].ap[1], [0, D], ee[:].ap[2]]
                        )
                        nc.vector.tensor_tensor(wm[:], uht[:], eb, op=OP.mult)
                        with nc.allow_low_precision(reason="t bf16 validated"):
                            nc.vector.tensor_reduce(
                                tt[:], wm[:], axis=AX.X, op=OP.add
                            )
                    # q = |t|^2, den = se^2 + q, rden = 1/den
                    sq = sm_pool.tile([128, K, D], dt.float32, tag="sqv", name="sqv")
                    nc.vector.tensor_tensor(sq[:], tt[:], tt[:], op=OP.mult)
                    q = smt(K, "q")
                    nc.vector.tensor_reduce(q[:], sq[:], axis=AX.X, op=OP.add)
                    den = smt(K, "den")
                    if r == 0:
                        nc.vector.tensor_single_scalar(
                            den[:], q[:], float(N * N), op=OP.add
                        )
                    else:
                        se2 = smt(K, "se2")
                        nc.vector.tensor_mul(se2[:], se[:], se[:])
                        nc.vector.tensor_add(den[:], q[:], se2[:])

                    rden = smt(K, "rden")
                    nc.vector.reciprocal(rden[:], den[:])
                    if r < 2:
                        sm = sqrt_half(q[:], K)
                        alpha2 = smt(K, "alpha2")
                        nc.vector.tensor_mul(alpha2[:], sm, rden[:])
                        am = tmp_pool.tile(
                            [128, K, N, D], dt.bfloat16, tag="am", name="am"
                        )
                        tb = _ap(
                            tt[:], [tt[:].ap[0], tt[:].ap[1], [0, N], tt[:].ap[2]]
                        )
                        nc.vector.tensor_tensor(am[:], uh[:], tb, op=OP.mult)
                        dta = sm_pool.tile(
                            [128, K, N], dt.bfloat16, tag="dta", name="dta"
                        )
                        with nc.allow_low_precision(reason="dta bf16 validated"):
                            nc.vector.tensor_reduce(
                                dta[:], am[:], axis=AX.X, op=OP.add
                            )
                        ab = _ap(alpha2[:], [*alpha2[:].ap, [0, N]])
                        if r == 0:
                            nc.gpsimd.tensor_tensor(bb[:], dta[:], ab, op=OP.mult)
                        else:
                            badd = sm_pool.tile(
                                [128, K, N], dt.float32, tag="badd", name="badd"
                            )
                            nc.gpsimd.tensor_tensor(badd[:], dta[:], ab, op=OP.mult)
                            nc.gpsimd.tensor_tensor(
                                bb[:], bb[:], badd[:], op=OP.add
                            )
                    else:
                        c0 = chs[0]
                        nc.vector.tensor_mul(out_sb[:, c0 : c0 + K], q[:], rden[:])
                        nc.gpsimd.dma_start(
                            out=out_ap[:, c0 : c0 + K],
                            in_=out_sb[:, c0 : c0 + K],
                        )

            for sp in range(NSP):
                xs = xs_pool.tile([128, NH, SUP], dt.bfloat16)
                for qd in range(NQ):
                    nc.sync.dma_start(
                        out=xs[:, qd * HQ : (qd + 1) * HQ, :],
                        in_=xt_ap[sp, qd, :, :],
                    )
                psp = psp_pool.tile([128, SUP], dt.float32)
                # zero-weight fillers: keep PE clocked up while DMA streams
                nwarm = n_warm if sp == 0 else n_fill
                for i in range(nwarm):
                    nc.tensor.matmul(
                        psp[:], zeros_sb[:], w4, start=(i == 0), stop=False
                    )
                ones_bc = _ap(
                    bias_sb[:, 128:256],
                    [bias_sb[:, 128:256].ap[0], [0, CPS], [1, 128]],
                )
                nc.tensor.matmul(
                    psp[:], bias_sb[:, 0:128], ones_bc, start=False, stop=False
                )
                for g in range(NQ):
                    for h in range(g * HQ, (g + 1) * HQ):
                        nc.tensor.matmul(
                            psp[:],
                            w_sb[:, h, :],
                            xs[:, h, :],
                            start=False,
                            stop=(h == NH - 1),
                        )
                    if g < NQ - 1:
                        for i in range(n_fill):
                            nc.tensor.matmul(
                                psp[:], zeros_sb[:], w4, start=False, stop=False
                            )
                primt = primt_pool.tile([128, SUP], dt.bfloat16)
                nc.scalar.copy(primt[:], psp[:])

                for c in range(CPS):
                    s = sp * CPS + c
                    bi = chunk_to_batch[s]
                    k = s - batches[bi][0]
                    lhsT = primt[:, c * 128 : (c + 1) * 128]
                    psa = psa_pool.tile([128, N * D], dt.float32)
                    nc.tensor.matmul(
                        psa[:], lhsT, capsn_sb[:], start=True, stop=True
                    )
                    psb = psb_pool.tile([128, D * N], dt.float32)
                    nc.tensor.matmul(
                        psb[:], lhsT, capst_sb[:], start=True, stop=True
                    )
                    psc = psc_pool.tile([128, D], dt.float32)
                    nc.tensor.matmul(
                        psc[:], lhsT, capsum_sb[:], start=True, stop=True
                    )
                    nc.scalar.copy(
                        uh_all[bi][:, k].rearrange("p n d -> p (n d)"), psa[:]
                    )
                    nc.scalar.copy(
                        uht_all[bi][:, k].rearrange("p d n -> p (d n)"), psb[:]
                    )
                    nc.scalar.copy(t_all[bi][:, k], psc[:])

                    if s in last_chunk_to_batch:
                        routing_batch(last_chunk_to_batch[s])

    nc.compile()
    return nc


def _prep_params(W, b_lin, out_caps, hidden=HIDDEN):
    NH = hidden // 128
    w2 = np.ascontiguousarray(
        W.astype(np.float32)
        .reshape(NH, 128, NUM_CAPS * CAP_DIM)
        .transpose(1, 0, 2)
        .reshape(128, NH * 128)
    ).astype(BF16)
    caps_bd = np.zeros((NUM_OBJ, 128, 128), np.float32)
    for o in range(NUM_OBJ):
        for i in range(NUM_CAPS):
            caps_bd[
                o, i * CAP_DIM : (i + 1) * CAP_DIM, i * CAP_DIM : (i + 1) * CAP_DIM
            ] = out_caps[o, i]
    capsn = np.ascontiguousarray(
        caps_bd.transpose(1, 0, 2).reshape(128, NUM_OBJ * 128)
    ).astype(BF16)
    capst = np.zeros((128, CAP_DIM, N_ROUTE), np.float32)
    for o in range(NUM_OBJ):
        for i in range(NUM_CAPS):
            capst[i * CAP_DIM : (i + 1) * CAP_DIM, :, o * NUM_CAPS + i] = out_caps[
                o, i
            ]
    capst = np.ascontiguousarray(capst.reshape(128, CAP_DIM * N_ROUTE)).astype(BF16)
    capsum = caps_bd.sum(0)
    capsum_t0 = np.zeros((128, CAP_DIM), np.float32)
    for i in range(NUM_CAPS):
        capsum_t0[i * CAP_DIM : (i + 1) * CAP_DIM, :] = capsum[
            i * CAP_DIM : (i + 1) * CAP_DIM, i * CAP_DIM : (i + 1) * CAP_DIM
        ]
    bias_row = np.concatenate(
        [
            b_lin.astype(np.float32).reshape(1, 128),
            np.ones((1, 128), np.float32),
        ],
        axis=1,
    )
    return (
        w2,
        capsn,
        capst,
        np.ascontiguousarray(capsum_t0).astype(BF16),
        bias_row.astype(BF16),
    )


_NC_CACHE = {}


def kernel(x, W, b_lin, out_caps):
    global LAST_EXEC_TIME_NS
    from concourse.bass_utils import run_bass_kernel_spmd

    x = np.asarray(x)
    W = np.asarray(W)
    b_lin = np.asarray(b_lin)
    out_caps = np.asarray(out_caps)
    bsz, hidden = x.shape
    b_sh = bsz // N_CORES
    NH = hidden // 128
    SUP = 512
    NSP = b_sh // SUP
    NQ = 4
    HQ = NH // NQ

    key = (hidden, b_sh)
    if key not in _NC_CACHE:
        _NC_CACHE[key] = build_bass(hidden=hidden, b_sh=b_sh)
    nc = _NC_CACHE[key]

    w2, capsn, capst, capsum_t0, bias_row = _prep_params(W, b_lin, out_caps, hidden)

    in_maps = []
    for i in range(N_CORES):
        shard = x[i * b_sh : (i + 1) * b_sh]
        # [sp, q, p, hc, b]: one contiguous 8KB DRAM line per (sp, q, p)
        xt = (
            shard.reshape(NSP, SUP, NQ, HQ, 128)
            .transpose(0, 2, 4, 3, 1)
            .astype(BF16)
            .reshape(NSP, NQ, 128, HQ * SUP)
        )
        in_maps.append(
            {
                "xt": np.ascontiguousarray(xt),
                "w": w2,
                "capsn": capsn,
                "capst": capst,
                "capsum": capsum_t0,
                "bias": bias_row,
            }
        )

    res = run_bass_kernel_spmd(
        nc,
        in_maps,
        core_ids=list(range(N_CORES)),
        trace=bool(int(os.environ.get("BASS_TRACE", "0") or "0")),
    )
    LAST_EXEC_TIME_NS = res.exec_time_ns
    return np.concatenate(
        [
            np.ascontiguousarray(res.results[i]["out"].T).reshape(-1)
            for i in range(N_CORES)
        ]
    )


# revision 11
# speedup vs baseline: 1.0844x; 1.0844x over previous
"""CapsuleRewardHead Trainium2 kernel (8-core data parallel).

Math (per batch row b):
    primary = x @ W + b_lin                    [B, 128]  (128 = 8 caps x 16 dim)
    u_hat[b,o,i,j] = sum_c primary[b,i,c] * out_caps[o,i,c,j]
    3 rounds of dynamic routing over N=32 capsule pairs (o,i), D=16
    out[b] = |squash(s_final)|

Device strategy per core (2048 batch rows):
  - host pre-converts the x shard to bf16 in a super-contiguous layout
    (one 8KB DRAM line per (super, h-slice, partition)), halving HBM
    traffic vs fp32 and keeping DMA descriptor-lean. 16 h-sliced loads
    are issued from the Sync engine so the stream is never queued behind
    data-dependent work.
  - MM1 (PE): primaryT[ic, b] += W.T @ xT over 32 h-chunks into PSUM;
    the Linear bias rides as a K=1 matmul against ones. Zero-weight
    filler matmuls pad PE idle gaps so the clock stays at full p-state
    (a cold/idle PE runs 512-col matmuls at ~427ns instead of ~216ns).
  - MM2 (PE): three matmuls per 128-row chunk produce u_hat in BOTH
    (n,d) and (d,n) layouts plus the round-0 uniform sum t0 (via
    column-permuted / summed copies of the block-diag capsule matrix),
    so every routing reduce is a unit-stride innermost-axis DVE reduce.
  - routing: e = exp(b) is stored compact [p,k,n] (bf16) and broadcast
    into the weighted-sum multiply via a 0-stride middle axis; t is
    kept bf16 so the agreement multiply's broadcast operand preserves
    DVE 2x mode. Unnormalized accumulators (q = |t|^2, se = sum e):
    alpha = sqrt(q)/(se^2+q), out = q/(se^2+q); sqrt via bit-trick.
    Engine split: DVE = big mults+reduces, ACT = exp + uh/t0/primt
    drains, GpSimd = uhT drains + logit updates + output DMA.
"""

import os

import numpy as np
import ml_dtypes

B = 16384
HIDDEN = 4096
NUM_OBJ = 4
NUM_CAPS = 8
CAP_DIM = 16
N_ROUTE = 32  # NUM_OBJ * NUM_CAPS
N_CORES = 8

LAST_EXEC_TIME_NS = None  # set after each run when BASS_TRACE=1

BF16 = ml_dtypes.bfloat16
SQRT_MAGIC = 0x1FBD1DF5


def _ap(ap, dims):
    import concourse.bass as bass

    return bass.AP(tensor=ap.tensor, offset=ap.offset, ap=dims)


def build_bass(
    hidden=HIDDEN,
    b_sh=B // N_CORES,
    batch_plan=(4, 4, 4, 2, 1, 1),
    n_warm=16,
    n_fill=3,
    use_accum=False,
    gps_halve=True,
):
    import concourse.tile as tile
    from concourse import bacc, mybir

    NH = hidden // 128  # 32 h-chunks
    SUP = 512
    NSP = b_sh // SUP  # supers
    CPS = SUP // 128  # chunks per super
    NCH = b_sh // 128  # 16 chunks
    NQ = 4  # h-slices per super
    HQ = NH // NQ
    N, D = N_ROUTE, CAP_DIM
    dt = mybir.dt
    AX = mybir.AxisListType
    OP = mybir.AluOpType
    AF = mybir.ActivationFunctionType
    assert sum(batch_plan) == NCH

    batches = []
    pos = 0
    for k in batch_plan:
        batches.append(list(range(pos, pos + k)))
        pos += k
    last_chunk_to_batch = {b[-1]: bi for bi, b in enumerate(batches)}
    chunk_to_batch = {}
    for bi, chs in enumerate(batches):
        for ch in chs:
            chunk_to_batch[ch] = bi

    nc = bacc.Bacc("TRN2", target_bir_lowering=False, debug=False, num_devices=N_CORES)

    xt_ap = nc.dram_tensor(
        "xt", [NSP, NQ, 128, HQ * SUP], dt.bfloat16, kind="ExternalInput"
    ).ap()
    w_ap = nc.dram_tensor("w", [128, NH * 128], dt.bfloat16, kind="ExternalInput").ap()
    capsn_ap = nc.dram_tensor(
        "capsn", [128, NUM_OBJ * 128], dt.bfloat16, kind="ExternalInput"
    ).ap()
    capst_ap = nc.dram_tensor(
        "capst", [128, D * N], dt.bfloat16, kind="ExternalInput"
    ).ap()
    capsum_ap = nc.dram_tensor(
        "capsum", [128, D], dt.bfloat16, kind="ExternalInput"
    ).ap()
    bias_ap = nc.dram_tensor("bias", [1, 256], dt.bfloat16, kind="ExternalInput").ap()
    out_ap = nc.dram_tensor("out", [128, NCH], dt.float32, kind="ExternalOutput").ap()

    with tile.TileContext(nc) as tc:
        with (
            tc.tile_pool(name="singles", bufs=1) as singles,
            tc.tile_pool(name="xs", bufs=3) as xs_pool,
            tc.tile_pool(name="primt", bufs=2) as primt_pool,
            tc.tile_pool(name="batch", bufs=1) as bpool,
            tc.tile_pool(name="tmp", bufs=2) as tmp_pool,
            tc.tile_pool(name="sm", bufs=8) as sm_pool,
            tc.tile_pool(name="psum_p", bufs=2, space="PSUM") as psp_pool,
            tc.tile_pool(name="psum_a", bufs=2, space="PSUM") as psa_pool,
            tc.tile_pool(name="psum_b", bufs=2, space="PSUM") as psb_pool,
            tc.tile_pool(name="psum_c", bufs=2, space="PSUM") as psc_pool,
        ):
            w_sb = singles.tile([128, NH, 128], dt.bfloat16)
            # split so the warmup (reading h<4) can start before the full
            # 1MB weight load completes
            nc.sync.dma_start(
                out=w_sb.rearrange("p h f -> p (h f)")[:, 0:512],
                in_=w_ap[:, 0:512],
            )
            nc.sync.dma_start(
                out=w_sb.rearrange("p h f -> p (h f)")[:, 512:],
                in_=w_ap[:, 512:],
            )
            capsn_sb = singles.tile([128, NUM_OBJ * 128], dt.bfloat16)
            nc.sync.dma_start(out=capsn_sb[:], in_=capsn_ap[:, :])
            capst_sb = singles.tile([128, D * N], dt.bfloat16)
            nc.sync.dma_start(out=capst_sb[:], in_=capst_ap[:, :])
            capsum_sb = singles.tile([128, D], dt.bfloat16)
            nc.sync.dma_start(out=capsum_sb[:], in_=capsum_ap[:, :])
            bias_sb = singles.tile([1, 256], dt.bfloat16)
            nc.sync.dma_start(out=bias_sb[:], in_=bias_ap[:, :])
            zeros_sb = singles.tile([128, 128], dt.bfloat16)
            nc.vector.memset(zeros_sb[:], 0)
            magic_sb = singles.tile([128, 1], dt.uint32)
            nc.vector.memset(magic_sb[:], SQRT_MAGIC)
            neg40_sb = singles.tile([128, 1], dt.float32)
            nc.vector.memset(neg40_sb[:], -40.0)
            out_sb = singles.tile([128, NCH], dt.float32)

            w4 = w_sb.rearrange("p h f -> p (h f)")[:, 0:512]

            uh_all, uht_all, t_all, b_all = {}, {}, {}, {}
            for bi, chs in enumerate(batches):
                K = len(chs)
                uh_all[bi] = bpool.tile(
                    [128, K, N, D], dt.bfloat16, tag=f"uh{bi}", name=f"uh{bi}"
                )
                uht_all[bi] = bpool.tile(
                    [128, K, D, N], dt.bfloat16, tag=f"uht{bi}", name=f"uht{bi}"
                )
                t_all[bi] = bpool.tile(
                    [128, K, D], dt.bfloat16, tag=f"t{bi}", name=f"t{bi}"
                )
                b_all[bi] = bpool.tile(
                    [128, K, N], dt.float32, tag=f"b{bi}", name=f"b{bi}"
                )

            def smt(K, tag, dtype=dt.float32):
                return sm_pool.tile([128, K], dtype, tag=tag, name=tag)

            def sqrt_half(q, K):
                """bit-trick sqrt seed; error washes out through squash."""
                qu = q.bitcast(dt.uint32)
                s1 = smt(K, "sq1", dt.uint32)
                nc.vector.tensor_single_scalar(
                    s1[:], qu, 1, op=OP.logical_shift_right
                )
                s2 = smt(K, "sq2", dt.uint32)
                nc.vector.tensor_tensor(
                    s2[:],
                    s1[:],
                    _ap(magic_sb[:], [magic_sb[:].ap[0], [0, K]]),
                    op=OP.add,
                )
                return s2.bitcast(dt.float32)  # ~3.5% sqrt approx (validated)

            def halved_reduce(out_ap_, src, K, outer, inner, tag, engine):
                """one tensor_tensor halving level (2x / offloadable) before
                the 1x-only tensor_reduce."""
                half = inner // 2
                h = tmp_pool.tile(
                    [128, K, outer, half],
                    dt.bfloat16,
                    tag=tag,
                    name=tag,
                )
                # src is [128, K, outer, inner]; halve the innermost axis
                engine.tensor_tensor(
                    h[:], src[:, :, :, 0:half], src[:, :, :, half:inner], op=OP.add
                )
                with nc.allow_low_precision(reason="bf16 reduce validated"):
                    nc.vector.tensor_reduce(out_ap_, h[:], axis=AX.X, op=OP.add)

            def routing_round(bi, r):
                chs = batches[bi]
                K = len(chs)
                uh = uh_all[bi]
                uht = uht_all[bi]
                tt = t_all[bi]
                bb = b_all[bi]
                big = K >= 4  # offload the halving level to GpSimd
                se = smt(K, "se")
                if r > 0:
                    # r2 logits can reach ~56 so se^2 would overflow fp32;
                    # bias the exp by -40 (cancels in q/(se^2+q), and the
                    # max logit is provably >= 0 so e^(max-40) stays normal).
                    ee = sm_pool.tile([128, K, N], dt.bfloat16, tag="ee", name="ee")
                    ebias = neg40_sb[:] if r == 2 else 0.0
                    if K == 1 and use_accum:
                        nc.scalar.activation(
                            ee[:], bb[:], AF.Exp, bias=ebias, accum_out=se[:]
                        )
                    else:
                        nc.scalar.activation(ee[:], bb[:], AF.Exp, bias=ebias)
                        nc.vector.tensor_reduce(se[:], ee[:], axis=AX.X, op=OP.add)
                    wm = tmp_pool.tile(
                        [128, K, D, N], dt.bfloat16, tag="wm", name="wm"
                    )
                    eb = _ap(ee[:], [ee[:].ap[0], ee[:].ap[1], [0, D], ee[:].ap[2]])
                    nc.vector.tensor_tensor(wm[:], uht[:], eb, op=OP.mult)
                    if K == 1:
                        with nc.allow_low_precision(reason="t bf16 validated"):
                            nc.vector.tensor_reduce(
                                tt[:], wm[:], axis=AX.X, op=OP.add
                            )
                    else:
                        halved_reduce(
                            tt[:], wm, K, D, N, "wmh",
                            nc.gpsimd if (big and gps_halve) else nc.vector,
                        )
                # q = |t|^2, den = se^2 + q, rden = 1/den
                q = smt(K, "q")
                if K == 1 and use_accum:
                    sq = sm_pool.tile([128, D], dt.float32, tag="sqv1", name="sqv1")
                    nc.vector.tensor_tensor_reduce(
                        sq[:], tt.rearrange("p k d -> p (k d)"),
                        tt.rearrange("p k d -> p (k d)"),
                        scale=1.0, scalar=0.0, op0=OP.mult, op1=OP.add,
                        accum_out=q[:],
                    )
                else:
                    sq = sm_pool.tile([128, K, D], dt.float32, tag="sqv", name="sqv")
                    nc.vector.tensor_tensor(sq[:], tt[:], tt[:], op=OP.mult)
                    nc.vector.tensor_reduce(q[:], sq[:], axis=AX.X, op=OP.add)
                den = smt(K, "den")
                if r == 0:
                    nc.vector.tensor_single_scalar(
                        den[:], q[:], float(N * N), op=OP.add
                    )
                else:
                    se2 = smt(K, "se2")
                    nc.vector.tensor_mul(se2[:], se[:], se[:])
                    nc.vector.tensor_add(den[:], q[:], se2[:])

                rden = smt(K, "rden")
                nc.vector.reciprocal(rden[:], den[:])
                if r < 2:
                    sm = sqrt_half(q[:], K)
                    alpha2 = smt(K, "alpha2")
                    nc.vector.tensor_mul(alpha2[:], sm, rden[:])
                    am = tmp_pool.tile(
                        [128, K, N, D], dt.bfloat16, tag="am", name="am"
                    )
                    tb = _ap(tt[:], [tt[:].ap[0], tt[:].ap[1], [0, N], tt[:].ap[2]])
                    nc.vector.tensor_tensor(am[:], uh[:], tb, op=OP.mult)
                    dta = sm_pool.tile([128, K, N], dt.bfloat16, tag="dta", name="dta")
                    if K == 1:
                        with nc.allow_low_precision(reason="dta bf16 validated"):
                            nc.vector.tensor_reduce(
                                dta[:], am[:], axis=AX.X, op=OP.add
                            )
                    else:
                        halved_reduce(
                            dta[:], am, K, N, D, "amh",
                            nc.gpsimd if (big and gps_halve) else nc.vector,
                        )
                    ab = _ap(alpha2[:], [*alpha2[:].ap, [0, N]])
                    if r == 0:
                        nc.vector.tensor_tensor(bb[:], dta[:], ab, op=OP.mult)
                    else:
                        badd = sm_pool.tile(
                            [128, K, N], dt.float32, tag="badd", name="badd"
                        )
                        nc.vector.tensor_tensor(badd[:], dta[:], ab, op=OP.mult)
                        nc.vector.tensor_add(bb[:], bb[:], badd[:])
                else:
                    c0 = chs[0]
                    nc.vector.tensor_mul(out_sb[:, c0 : c0 + K], q[:], rden[:])
                    nc.gpsimd.dma_start(
                        out=out_ap[:, c0 : c0 + K],
                        in_=out_sb[:, c0 : c0 + K],
                    )

            for sp in range(NSP):
                xs = xs_pool.tile([128, NH, SUP], dt.bfloat16)
                for qd in range(NQ):
                    nc.sync.dma_start(
                        out=xs[:, qd * HQ : (qd + 1) * HQ, :],
                        in_=xt_ap[sp, qd, :, :],
                    )
                psp = psp_pool.tile([128, SUP], dt.float32)
                # zero-weight fillers: keep PE clocked up while DMA streams
                nwarm = n_warm if sp == 0 else n_fill
                for i in range(nwarm):
                    nc.tensor.matmul(
                        psp[:], zeros_sb[:], w4, start=(i == 0), stop=False
                    )
                ones_bc = _ap(
                    bias_sb[:, 128:256],
                    [bias_sb[:, 128:256].ap[0], [0, CPS], [1, 128]],
                )
                nc.tensor.matmul(
                    psp[:], bias_sb[:, 0:128], ones_bc, start=False, stop=False
                )
                for g in range(NQ):
                    for h in range(g * HQ, (g + 1) * HQ):
                        nc.tensor.matmul(
                            psp[:],
                            w_sb[:, h, :],
                            xs[:, h, :],
                            start=False,
                            stop=(h == NH - 1),
                        )
                    if g < NQ - 1:
                        for i in range(n_fill):
                            nc.tensor.matmul(
                                psp[:], zeros_sb[:], w4, start=False, stop=False
                            )
                primt = primt_pool.tile([128, SUP], dt.bfloat16)
                nc.scalar.copy(primt[:], psp[:])

                for c in range(CPS):
                    s = sp * CPS + c
                    bi = chunk_to_batch[s]
                    k = s - batches[bi][0]
                    lhsT = primt[:, c * 128 : (c + 1) * 128]
                    psa = psa_pool.tile([128, N * D], dt.float32)
                    nc.tensor.matmul(
                        psa[:], lhsT, capsn_sb[:], start=True, stop=True
                    )
                    psb = psb_pool.tile([128, D * N], dt.float32)
                    nc.tensor.matmul(
                        psb[:], lhsT, capst_sb[:], start=True, stop=True
                    )
                    psc = psc_pool.tile([128, D], dt.float32)
                    nc.tensor.matmul(
                        psc[:], lhsT, capsum_sb[:], start=True, stop=True
                    )
                    nc.scalar.copy(
                        uh_all[bi][:, k].rearrange("p n d -> p (n d)"), psa[:]
                    )
                    nc.scalar.copy(
                        uht_all[bi][:, k].rearrange("p d n -> p (d n)"), psb[:]
                    )
                    nc.scalar.copy(t_all[bi][:, k], psc[:])

                    if s in last_chunk_to_batch:
                        bi2 = last_chunk_to_batch[s]
                        if batches[bi2][-1] < (NSP - 1) * CPS:
                            # mid-stream batch: full 3-round chain
                            for r in range(3):
                                routing_round(bi2, r)
                        else:
                            # last-super batch: round 0 now, later rounds
                            # interleaved across tail batches (below) so
                            # engine queues overlap the serial chains
                            routing_round(bi2, 0)
                    if s == NCH - 1:
                        tail = [
                            bi2
                            for bi2, chs2 in enumerate(batches)
                            if chs2[-1] >= (NSP - 1) * CPS
                        ]
                        for r in (1, 2):
                            for bi2 in tail:
                                routing_round(bi2, r)

    nc.compile()
    return nc


def _prep_params(W, b_lin, out_caps, hidden=HIDDEN):
    NH = hidden // 128
    w2 = np.ascontiguousarray(
        W.astype(np.float32)
        .reshape(NH, 128, NUM_CAPS * CAP_DIM)
        .transpose(1, 0, 2)
        .reshape(128, NH * 128)
    ).astype(BF16)
    caps_bd = np.zeros((NUM_OBJ, 128, 128), np.float32)
    for o in range(NUM_OBJ):
        for i in range(NUM_CAPS):
            caps_bd[
                o, i * CAP_DIM : (i + 1) * CAP_DIM, i * CAP_DIM : (i + 1) * CAP_DIM
            ] = out_caps[o, i]
    capsn = np.ascontiguousarray(
        caps_bd.transpose(1, 0, 2).reshape(128, NUM_OBJ * 128)
    ).astype(BF16)
    capst = np.zeros((128, CAP_DIM, N_ROUTE), np.float32)
    for o in range(NUM_OBJ):
        for i in range(NUM_CAPS):
            capst[i * CAP_DIM : (i + 1) * CAP_DIM, :, o * NUM_CAPS + i] = out_caps[
                o, i
            ]
    capst = np.ascontiguousarray(capst.reshape(128, CAP_DIM * N_ROUTE)).astype(BF16)
    capsum = caps_bd.sum(0)
    capsum_t0 = np.zeros((128, CAP_DIM), np.float32)
    for i in range(NUM_CAPS):
        capsum_t0[i * CAP_DIM : (i + 1) * CAP_DIM, :] = capsum[
            i * CAP_DIM : (i + 1) * CAP_DIM, i * CAP_DIM : (i + 1) * CAP_DIM
        ]
    bias_row = np.concatenate(
        [
            b_lin.astype(np.float32).reshape(1, 128),
            np.ones((1, 128), np.float32),
        ],
        axis=1,
    )
    return (
        w2,
        capsn,
        capst,
        np.ascontiguousarray(capsum_t0).astype(BF16),
        bias_row.astype(BF16),
    )


_NC_CACHE = {}


def kernel(x, W, b_lin, out_caps):
    global LAST_EXEC_TIME_NS
    from concourse.bass_utils import run_bass_kernel_spmd

    x = np.asarray(x)
    W = np.asarray(W)
    b_lin = np.asarray(b_lin)
    out_caps = np.asarray(out_caps)
    bsz, hidden = x.shape
    b_sh = bsz // N_CORES
    NH = hidden // 128
    SUP = 512
    NSP = b_sh // SUP
    NQ = 4
    HQ = NH // NQ

    key = (hidden, b_sh)
    if key not in _NC_CACHE:
        _NC_CACHE[key] = build_bass(
            hidden=hidden,
            b_sh=b_sh,
            use_accum=bool(int(os.environ.get("K_ACCUM", "0"))),
            gps_halve=bool(int(os.environ.get("K_GPSH", "1"))),
        )
    nc = _NC_CACHE[key]

    w2, capsn, capst, capsum_t0, bias_row = _prep_params(W, b_lin, out_caps, hidden)

    in_maps = []
    for i in range(N_CORES):
        shard = x[i * b_sh : (i + 1) * b_sh]
        # [sp, q, p, hc, b]: one contiguous 8KB DRAM line per (sp, q, p)
        xt = (
            shard.reshape(NSP, SUP, NQ, HQ, 128)
            .transpose(0, 2, 4, 3, 1)
            .astype(BF16)
            .reshape(NSP, NQ, 128, HQ * SUP)
        )
        in_maps.append(
            {
                "xt": np.ascontiguousarray(xt),
                "w": w2,
                "capsn": capsn,
                "capst": capst,
                "capsum": capsum_t0,
                "bias": bias_row,
            }
        )

    res = run_bass_kernel_spmd(
        nc,
        in_maps,
        core_ids=list(range(N_CORES)),
        trace=bool(int(os.environ.get("BASS_TRACE", "0") or "0")),
    )
    LAST_EXEC_TIME_NS = res.exec_time_ns
    return np.concatenate(
        [
            np.ascontiguousarray(res.results[i]["out"].T).reshape(-1)
            for i in range(N_CORES)
        ]
    )
